# revision 4
# baseline (speedup 1.0000x reference)
"""Trainium2 Bass kernel for CoocOpModel.

out[b,s,z] = sum_{i,j} func[b,s,i] * cooc[i,j,z] * arg[b,s,j]
  with func = func_and_arg[..., :128], arg = func_and_arg[..., 128:]

Shapes (hardcoded): func_and_arg [4,1024,256] f32, cooccurrences [128,128,128] f32,
out [4,1024,128] f32.  D = 128, tokens T = 4096.

Strategy: data-parallel over tokens across 8 cores (512 tokens/core);
cooccurrence tensor replicated per core (fp16).

Per-core math, with t = local token index (512), i/j/z in [0,128):
  out_T[z, t] = sum_i  C_i^T @ G_i        (accumulated in one PSUM bank)
  C_i[j, z]   = cooc[i, j, z]             (stationary operand, fp16)
  G_i[j, t]   = arg_T[j, t] * func_T[i, t]  (moving operand, fp16)

i's are processed in groups.  For each group we need
f_exp[j, (k,t)] = func_T[i0+k, t] replicated across the 128 j-partitions,
then one DVE tensor-tensor multiply builds G for the group and the
per-i matmuls accumulate into PSUM.

The f_exp replication (D*D*T_core*2B = 16 MB/core) dominated DMA when done
purely as a broadcast-DMA from DRAM (DMA fabric ~358 GB/s/core was the
bottleneck at ~59us busy).  So the replication is SPLIT:
  - 'd' groups: broadcast-DMA from DRAM (partition-step-0 source AP)
  - 'p' groups: PE ones-matmul (stationary ones[1,128], moving f row
    chunks living on a single SBUF partition) writes the replicated rows
    into PSUM; the otherwise-idle Activation engine copies PSUM fp32 ->
    SBUF bf16.  This moves broadcast bytes off the saturated DMA fabric
    onto idle PE columns + ACT.
"""

import sys

sys.path.insert(0, "/opt/trn_rl_repo")

import numpy as np
from contextlib import ExitStack

import concourse.bass as bass
import concourse.tile as tile
from concourse import bacc, mybir
from concourse.bass_utils import run_bass_kernel_spmd

BF16 = mybir.dt.float16
F32 = mybir.dt.float32
NP_BF16 = np.float16

N_CORES = 8
D = 128
T_TOTAL = 4096
T_CORE = T_TOTAL // N_CORES  # 512

# Execution schedule: list of (kind, n_rows). 'd' = DMA-broadcast group,
# 'p' = PE+ACT broadcast group. Sum of n_rows must be 128.
N_PE = 7
if N_PE == 0:
    SCHED = [("d", 4), ("d", 4)] + [("d", 8)] * 14 + [("d", 4), ("d", 4)]
else:
    SCHED = [("d", 4), ("d", 4)]
    _d_rows = 128 - 8 * N_PE - 16  # rows for middle d-groups
    _mid = []
    _nd_mid = (_d_rows + 7) // 8
    rem = _d_rows
    for q in range(N_PE):
        _mid.append(("p", 8))
        if rem > 0:
            take = min(8, rem)
            _mid.append(("d", take))
            rem -= take
    assert rem == 0, rem
    SCHED += _mid + [("d", 4), ("d", 4)]
assert sum(r for _, r in SCHED) == D, SCHED

PE_ROWS = sum(r for k, r in SCHED if k == "p")
I_PE0 = D - PE_ROWS  # pe-groups take the top i-range [I_PE0, 128)
BC_CHUNK = 1024  # psum broadcast chunk (2 banks); built by 2x 512-col mms

_NC_CACHE = None


def _build():
    nc = bacc.Bacc("TRN2", target_bir_lowering=False, debug=False, num_devices=N_CORES)

    f_t = nc.dram_tensor("f_t", [D, T_CORE], BF16, kind="ExternalInput").ap()
    a_t = nc.dram_tensor("a_t", [D, T_CORE], BF16, kind="ExternalInput").ap()
    # c2[j, i*128 + z] = cooc[i, j, z]
    c2 = nc.dram_tensor("c2", [D, D * D], BF16, kind="ExternalInput").ap()
    out_t = nc.dram_tensor("out_t", [D, T_CORE], F32, kind="ExternalOutput").ap()

    with tile.TileContext(nc) as tc:
        with ExitStack() as ctx:
            const_pool = ctx.enter_context(tc.tile_pool(name="const", bufs=1))
            fexp_pool = ctx.enter_context(tc.tile_pool(name="fexp", bufs=4))
            fxp_pool = ctx.enter_context(tc.tile_pool(name="fxp", bufs=3))
            g_pool = ctx.enter_context(tc.tile_pool(name="g", bufs=3))
            out_pool = ctx.enter_context(tc.tile_pool(name="out", bufs=1))
            psum_pool = ctx.enter_context(
                tc.tile_pool(name="psum", bufs=1, space="PSUM")
            )
            bc_pool = ctx.enter_context(
                tc.tile_pool(name="bcps", bufs=3, space="PSUM")
            )

            # arg_T in SBUF; TTs re-read it per k via a free-step-0 AP.
            a_sb = const_pool.tile([D, T_CORE], BF16, tag="a")
            nc.gpsimd.dma_start(a_sb[:], a_t[:, :])
            a_ap = a_sb[:]

            if PE_ROWS:
                # ones row for the broadcast matmuls + the pe-groups' f rows,
                # all on SBUF partition 0 (moving operand of a 1-contract mm).
                ones_sb = const_pool.tile([1, D], BF16, tag="ones")
                nc.gpsimd.memset(ones_sb[:], 1.0)
                f_pe = const_pool.tile([1, PE_ROWS * T_CORE], BF16, tag="fpe")
                f_pe_src = bass.AP(
                    f_t.tensor, I_PE0 * T_CORE, [[0, 1], [1, PE_ROWS * T_CORE]]
                )
                nc.gpsimd.dma_start(f_pe[:], f_pe_src)

            ps = psum_pool.tile([D, T_CORE], F32)

            # Precompute per-position metadata: i0 (d-groups walk up from 0,
            # p-groups walk up from I_PE0).
            meta = []
            d_i0, p_i0 = 0, I_PE0
            for kind, sz in SCHED:
                if kind == "d":
                    meta.append((kind, sz, d_i0))
                    d_i0 += sz
                else:
                    meta.append((kind, sz, p_i0))
                    p_i0 += sz
            n_pos = len(meta)
            first_mm = True

            # Emission bookkeeping for pe-groups: broadcast mms + ACT copies
            # are emitted LOOKAHEAD positions before their consuming position
            # so the in-order PE queue never stalls on them.
            LOOKAHEAD = 2
            fexp_tiles = {}  # position -> f_exp sbuf tile (pe-groups)

            def emit_pe_broadcast(p):
                kind, sz, i0 = meta[p]
                assert kind == "p" and sz == 8
                fx = fxp_pool.tile([D, sz * T_CORE], BF16, tag="fxp")
                q_base = (i0 - I_PE0) * T_CORE
                n_chunk = sz * T_CORE // BC_CHUNK
                for c in range(n_chunk):
                    bc = bc_pool.tile([D, BC_CHUNK], F32, tag="bc")
                    for h in range(BC_CHUNK // 512):
                        off = q_base + c * BC_CHUNK + h * 512
                        nc.tensor.matmul(
                            bc[:, h * 512 : (h + 1) * 512],
                            ones_sb[:],
                            f_pe[:, off : off + 512],
                            start=True,
                            stop=True,
                        )
                    nc.scalar.copy(
                        fx[:, c * BC_CHUNK : (c + 1) * BC_CHUNK], bc[:]
                    )
                fexp_tiles[p] = fx

            dq = 0  # alternator for dma queues
            for p in range(n_pos):
                # early-emit pe broadcasts
                for pp in range(p, min(p + LOOKAHEAD + 1, n_pos)):
                    if meta[pp][0] == "p" and pp not in fexp_tiles:
                        emit_pe_broadcast(pp)

                kind, sz, i0 = meta[p]
                gt = g_pool.tile([D, sz * T_CORE], BF16, tag="g")

                # cooc tile for this group's i-range
                c_sb = const_pool.tile([D, sz * D], BF16, tag=f"c{p}")
                ceng = nc.sync if dq % 2 == 0 else nc.gpsimd
                ceng.dma_start(c_sb[:], c2[:, i0 * D : (i0 + sz) * D])

                if kind == "d":
                    fx = fexp_pool.tile([D, sz * T_CORE], BF16, tag="fxd")
                    if p == 0:
                        # split the head transfer across both queues
                        half = sz // 2
                        src_a = bass.AP(
                            f_t.tensor,
                            i0 * T_CORE,
                            [[0, D], [T_CORE, half], [1, T_CORE]],
                        )
                        src_b = bass.AP(
                            f_t.tensor,
                            (i0 + half) * T_CORE,
                            [[0, D], [T_CORE, half], [1, T_CORE]],
                        )
                        nc.sync.dma_start(fx[:, : half * T_CORE], src_a)
                        nc.gpsimd.dma_start(fx[:, half * T_CORE :], src_b)
                    else:
                        src = bass.AP(
                            f_t.tensor,
                            i0 * T_CORE,
                            [[0, D], [T_CORE, sz], [1, T_CORE]],
                        )
                        eng = nc.gpsimd if dq % 2 == 0 else nc.sync
                        eng.dma_start(fx[:], src)
                    dq += 1
                    a_view = bass.AP(
                        a_ap.tensor, a_ap.offset, [a_ap.ap[0], [0, sz], [1, T_CORE]]
                    )
                    nc.vector.tensor_mul(gt[:], a_view, fx[:])
                else:
                    fx = fexp_tiles[p]
                    # TT per broadcast chunk so DVE starts as ACT lands chunks
                    n_chunk = sz * T_CORE // BC_CHUNK
                    kpc = BC_CHUNK // T_CORE  # k's per chunk
                    a_view = bass.AP(
                        a_ap.tensor, a_ap.offset, [a_ap.ap[0], [0, kpc], [1, T_CORE]]
                    )
                    for c in range(n_chunk):
                        nc.vector.tensor_mul(
                            gt[:, c * BC_CHUNK : (c + 1) * BC_CHUNK],
                            a_view,
                            fx[:, c * BC_CHUNK : (c + 1) * BC_CHUNK],
                        )

                for k in range(sz):
                    last = (p == n_pos - 1) and (k == sz - 1)
                    nc.tensor.matmul(
                        ps[:],
                        c_sb[:, k * D : (k + 1) * D],
                        gt[:, k * T_CORE : (k + 1) * T_CORE],
                        start=first_mm,
                        stop=last,
                    )
                    first_mm = False

            # drain: split output so the two copy engines + DMA overlap
            o_sb = out_pool.tile([D, T_CORE], F32, tag="o")
            h = T_CORE // 2
            nc.vector.tensor_copy(o_sb[:, :h], ps[:, :h])
            nc.sync.dma_start(out_t[:, :h], o_sb[:, :h])
            nc.scalar.copy(o_sb[:, h:], ps[:, h:])
            nc.gpsimd.dma_start(out_t[:, h:], o_sb[:, h:])

    nc.compile()
    return nc


def _get_nc():
    global _NC_CACHE
    if _NC_CACHE is None:
        _NC_CACHE = _build()
    return _NC_CACHE


def _prep_in_maps(func_and_arg, cooccurrences):
    fa = np.asarray(func_and_arg, dtype=np.float32).reshape(T_TOTAL, 2 * D)
    c2 = (
        np.ascontiguousarray(
            np.asarray(cooccurrences, dtype=np.float32).transpose(1, 0, 2)
        )
        .reshape(D, D * D)
        .astype(NP_BF16)
    )
    in_maps = []
    for c in range(N_CORES):
        s = fa[c * T_CORE : (c + 1) * T_CORE]  # [512, 256]
        f_tc = np.ascontiguousarray(s[:, :D].T).astype(NP_BF16)  # [128 i, 512 t]
        a_tc = np.ascontiguousarray(s[:, D:].T).astype(NP_BF16)  # [128 j, 512 t]
        in_maps.append({"f_t": f_tc, "a_t": a_tc, "c2": c2})
    return in_maps


def kernel(func_and_arg: np.ndarray, cooccurrences: np.ndarray) -> np.ndarray:
    assert func_and_arg.shape == (4, 1024, 2 * D)
    assert cooccurrences.shape == (D, D, D)

    in_maps = _prep_in_maps(func_and_arg, cooccurrences)
    nc = _get_nc()
    res = run_bass_kernel_spmd(nc, in_maps, core_ids=list(range(N_CORES)))

    # out_t per core: [z=128, t=512] -> [t, z]; concat over cores -> [4096, 128]
    outs = [res.results[c]["out_t"].T for c in range(N_CORES)]
    out = np.concatenate(outs, axis=0).reshape(4, 1024, D).astype(np.float32)
    return out


# revision 7
# speedup vs baseline: 1.0101x; 1.0101x over previous
"""Trainium2 Bass kernel for CoocOpModel.

out[b,s,z] = sum_{i,j} func[b,s,i] * cooc[i,j,z] * arg[b,s,j]
  with func = func_and_arg[..., :128], arg = func_and_arg[..., 128:]

Shapes (hardcoded): func_and_arg [4,1024,256] f32, cooccurrences [128,128,128] f32,
out [4,1024,128] f32.  D = 128, tokens T = 4096.

Strategy: data-parallel over tokens across 8 cores (512 tokens/core);
cooccurrence tensor replicated per core (fp16).

Per-core math, with t = local token index (512), i/j/z in [0,128):
  out_T[z, t] = sum_i  C_i^T @ G_i        (accumulated in one PSUM bank)
  C_i[j, z]   = cooc[i, j, z]             (stationary operand, fp16)
  G_i[j, t]   = arg_T[j, t] * func_T[i, t]  (moving operand, fp16)

For each i-group we need f_exp[j, (k,t)] = func_T[i0+k, t] replicated
across the 128 j-partitions; a DVE tensor-tensor multiply then builds G
and the per-i matmuls accumulate into PSUM.

The replication (D*D*T_core*2B = 16 MB/core) saturates the per-core DMA
fabric (~358 GB/s) if done purely as broadcast-DMA from DRAM, so it is
split:
  - 'd' groups: broadcast-DMA from DRAM (partition-step-0 source AP)
  - 'p' groups: PE ones-matmul (stationary ones[1,128], moving = f row
    chunks on a single SBUF partition) replicates a 512-wide f row into a
    PSUM bank; the DVE multiplies straight out of PSUM (fp32 in1, 1x
    instead of 2x, but no extra DMA bytes).  The broadcast matmuls are
    interleaved between the accumulating matmuls so the in-order PE queue
    never waits on them.
"""

import sys

sys.path.insert(0, "/opt/trn_rl_repo")

import numpy as np
from contextlib import ExitStack

import concourse.bass as bass
import concourse.tile as tile
from concourse import bacc, mybir
from concourse.bass_utils import run_bass_kernel_spmd

BF16 = mybir.dt.float16
F32 = mybir.dt.float32
NP_BF16 = np.float16

N_CORES = 8
D = 128
T_TOTAL = 4096
T_CORE = T_TOTAL // N_CORES  # 512

# Execution schedule: ('d'|'p', n_rows); sum of rows = 128.
# p-groups sit early/middle; head and tail are small d-groups.
SCHED = [
    ("d", 4), ("d", 4),
    ("p", 8), ("d", 8), ("d", 8), ("d", 8),
    ("p", 8), ("d", 8), ("d", 8), ("d", 8),
    ("p", 8), ("d", 8), ("d", 8), ("d", 8),
    ("d", 8), ("d", 8), ("d", 4), ("d", 4),
]
assert sum(r for _, r in SCHED) == D
PE_ROWS = sum(r for k, r in SCHED if k == "p")
I_PE0 = D - PE_ROWS  # p-groups own the top i-range [I_PE0, 128)

_NC_CACHE = None


def _build():
    nc = bacc.Bacc("TRN2", target_bir_lowering=False, debug=False, num_devices=N_CORES)

    f_t = nc.dram_tensor("f_t", [D, T_CORE], BF16, kind="ExternalInput").ap()
    a_t = nc.dram_tensor("a_t", [D, T_CORE], BF16, kind="ExternalInput").ap()
    # c2[j, i*128 + z] = cooc[i, j, z]
    c2 = nc.dram_tensor("c2", [D, D * D], BF16, kind="ExternalInput").ap()
    out_t = nc.dram_tensor("out_t", [D, T_CORE], F32, kind="ExternalOutput").ap()

    with tile.TileContext(nc) as tc:
        with ExitStack() as ctx:
            const_pool = ctx.enter_context(tc.tile_pool(name="const", bufs=1))
            fexp_pool = ctx.enter_context(tc.tile_pool(name="fexp", bufs=4))
            g_pool = ctx.enter_context(tc.tile_pool(name="g", bufs=3))
            out_pool = ctx.enter_context(tc.tile_pool(name="out", bufs=1))
            psum_pool = ctx.enter_context(
                tc.tile_pool(name="psum", bufs=1, space="PSUM")
            )
            bc_pool = ctx.enter_context(
                tc.tile_pool(name="bcps", bufs=5, space="PSUM")
            )

            a_sb = const_pool.tile([D, T_CORE], BF16, tag="a")
            nc.gpsimd.dma_start(a_sb[:], a_t[:, :])
            a_ap = a_sb[:]

            if PE_ROWS:
                ones_sb = const_pool.tile([1, D], BF16, tag="ones")
                nc.gpsimd.memset(ones_sb[:], 1.0)
                f_pe = const_pool.tile([1, PE_ROWS * T_CORE], BF16, tag="fpe")
                f_pe_src = bass.AP(
                    f_t.tensor, I_PE0 * T_CORE, [[0, 1], [1, PE_ROWS * T_CORE]]
                )
                nc.sync.dma_start(f_pe[:], f_pe_src)

            ps = psum_pool.tile([D, T_CORE], F32)

            meta = []
            d_i0, p_i0 = 0, I_PE0
            for kind, sz in SCHED:
                if kind == "d":
                    meta.append((kind, sz, d_i0))
                    d_i0 += sz
                else:
                    meta.append((kind, sz, p_i0))
                    p_i0 += sz
            n_pos = len(meta)

            state = {"first": True}

            def real_mm(c_sb, gt, k, last):
                nc.tensor.matmul(
                    ps[:],
                    c_sb[:, k * D : (k + 1) * D],
                    gt[:, k * T_CORE : (k + 1) * T_CORE],
                    start=state["first"],
                    stop=last,
                )
                state["first"] = False

            def bc_mm(p, k):
                """Broadcast row i0+k of p-group at position p into a PSUM bank."""
                _, _, i0 = meta[p]
                off = (i0 - I_PE0) * T_CORE + k * T_CORE
                bc = bc_pool.tile([D, T_CORE], F32, tag="bc")
                nc.tensor.matmul(
                    bc[:],
                    ones_sb[:],
                    f_pe[:, off : off + T_CORE],
                    start=True,
                    stop=True,
                )
                return bc

            # bc tiles pending TT-consumption, keyed by position
            bc_tiles = {}

            dq = 0
            for p in range(n_pos):
                kind, sz, i0 = meta[p]
                nxt_pe = p + 1 < n_pos and meta[p + 1][0] == "p"

                c_sb = const_pool.tile([D, sz * D], BF16, tag=f"c{p}")
                ceng = nc.sync if dq % 2 == 0 else nc.gpsimd
                ceng.dma_start(c_sb[:], c2[:, i0 * D : (i0 + sz) * D])

                gt = g_pool.tile([D, sz * T_CORE], BF16, tag="g")

                if kind == "d":
                    fx = fexp_pool.tile([D, sz * T_CORE], BF16, tag="fxd")
                    if p == 0:
                        half = sz // 2
                        src_a = bass.AP(
                            f_t.tensor,
                            i0 * T_CORE,
                            [[0, D], [T_CORE, half], [1, T_CORE]],
                        )
                        src_b = bass.AP(
                            f_t.tensor,
                            (i0 + half) * T_CORE,
                            [[0, D], [T_CORE, half], [1, T_CORE]],
                        )
                        nc.sync.dma_start(fx[:, : half * T_CORE], src_a)
                        nc.gpsimd.dma_start(fx[:, half * T_CORE :], src_b)
                    else:
                        src = bass.AP(
                            f_t.tensor,
                            i0 * T_CORE,
                            [[0, D], [T_CORE, sz], [1, T_CORE]],
                        )
                        eng = nc.gpsimd if dq % 2 == 0 else nc.sync
                        eng.dma_start(fx[:], src)
                    dq += 1
                    a_view = bass.AP(
                        a_ap.tensor, a_ap.offset, [a_ap.ap[0], [0, sz], [1, T_CORE]]
                    )
                    nc.vector.tensor_mul(gt[:], a_view, fx[:])
                    # real mms; if the next position is a p-group, pre-issue
                    # its first 3 broadcast mms between our tail mms.
                    pre = 4 if nxt_pe else 0
                    lst = []
                    for k in range(sz):
                        real_mm(c_sb, gt, k, (p == n_pos - 1) and (k == sz - 1))
                        if pre and k >= sz - pre - 1 and len(lst) < pre:
                            lst.append(bc_mm(p + 1, len(lst)))
                    bc_tiles[p + 1] = lst
                else:
                    lst = bc_tiles.get(p, [])
                    # interleave: TT chunk k first (so pool reuse sees the
                    # read), then the bc mm for a later chunk, then real k.
                    for k in range(sz):
                        nc.vector.tensor_mul(
                            gt[:, k * T_CORE : (k + 1) * T_CORE],
                            a_ap,
                            lst[k][:],
                        )
                        if len(lst) < sz:
                            lst.append(bc_mm(p, len(lst)))
                        real_mm(c_sb, gt, k, False)

            # drain: split output halves across two copy engines + queues
            o_sb = out_pool.tile([D, T_CORE], F32, tag="o")
            h = T_CORE // 2
            nc.vector.tensor_copy(o_sb[:, :h], ps[:, :h])
            nc.sync.dma_start(out_t[:, :h], o_sb[:, :h])
            nc.scalar.copy(o_sb[:, h:], ps[:, h:])
            nc.gpsimd.dma_start(out_t[:, h:], o_sb[:, h:])

    nc.compile()
    return nc


def _get_nc():
    global _NC_CACHE
    if _NC_CACHE is None:
        _NC_CACHE = _build()
    return _NC_CACHE


def _prep_in_maps(func_and_arg, cooccurrences):
    fa = np.asarray(func_and_arg, dtype=np.float32).reshape(T_TOTAL, 2 * D)
    c2 = (
        np.ascontiguousarray(
            np.asarray(cooccurrences, dtype=np.float32).transpose(1, 0, 2)
        )
        .reshape(D, D * D)
        .astype(NP_BF16)
    )
    in_maps = []
    for c in range(N_CORES):
        s = fa[c * T_CORE : (c + 1) * T_CORE]  # [512, 256]
        f_tc = np.ascontiguousarray(s[:, :D].T).astype(NP_BF16)  # [128 i, 512 t]
        a_tc = np.ascontiguousarray(s[:, D:].T).astype(NP_BF16)  # [128 j, 512 t]
        in_maps.append({"f_t": f_tc, "a_t": a_tc, "c2": c2})
    return in_maps


def kernel(func_and_arg: np.ndarray, cooccurrences: np.ndarray) -> np.ndarray:
    assert func_and_arg.shape == (4, 1024, 2 * D)
    assert cooccurrences.shape == (D, D, D)

    in_maps = _prep_in_maps(func_and_arg, cooccurrences)
    nc = _get_nc()
    res = run_bass_kernel_spmd(nc, in_maps, core_ids=list(range(N_CORES)))

    # out_t per core: [z=128, t=512] -> [t, z]; concat over cores -> [4096, 128]
    outs = [res.results[c]["out_t"].T for c in range(N_CORES)]
    out = np.concatenate(outs, axis=0).reshape(4, 1024, D).astype(np.float32)
    return out
